# revision 32
# baseline (speedup 1.0000x reference)
"""Trainium2 Bass kernel for nn_Attention (AdderNet attention block).

Problem: B=8, S=197, E=384, H=6, D=64.
  x2d = x.reshape(E, B*S)                      # flat reshape, [384, 1576]
  per proj (q,k,v):  Y = -sum_ci |x2d[ci,n] - w[co,ci]|   (adder 1x1)
                     LN over ALL of [E,B,S] (elementwise affine params)
                     flat-reshape to [B,S,H,D] -> heads
  att = softmax(q k^T * scale) + I; o = att v; token-LN; fc.

Sharding: core c owns co-rows [48c, 48c+48) of each of the three adder
projections.  Because b*S*E == b*48*1576, those rows are exactly the
post-LN data needed for batch b=c of the attention, so phase B (attention
+ out-LN + fc) is fully local per core.  The only cross-core exchange is
a 6-float AllReduce for the global LayerNorm statistics.

On-core algorithm (phase A): layout partition=ci, free=n.  x rows are
replicated 4x along partitions ([4co x 32ci] blocks); a fused abs op
(ACT: Abs(x + (-w)) with per-partition bias, or DVE: tensor_scalar
(x + (-w)) abs_max 0) produces |x-w| tiles which the PE reduces over ci
with a block-ones stationary matrix into PSUM (fp32 bitcast to float32r:
full-rate at N>=256).  36 groups x 12 chunks -> [4, 1576] PSUM rows,
DMA-evacuated to DRAM.
"""

import numpy as np
from contextlib import ExitStack

B, S, E = 8, 197, 384
H, D = 6, 64
N = B * S            # 1576
RPC = E // 8         # 48 rows per core per projection
NCORE = 8
NTOT = E * N         # 605184 elements per projection
C_SHIFT = 307.0      # conditioning shift for sum-of-squares (A ~ +307)
EPS = 1e-5
SCALE = float((2.0 * D * (1.0 - 2.0 / np.pi)) ** (-0.5))
NCH = [(0, 512), (512, 1024), (1024, 1536), (1536, 1576)]
SBLK = [(0, 128), (128, 197)]     # token blocks of 197
EBLK = [(0, 128), (128, 256), (256, 384)]

_PROGRAM = None


def _build_program():
    import concourse.bass as bass
    import concourse.mybir as mybir
    from concourse import bacc, tile

    dt = mybir.dt
    f32 = dt.float32
    bf16 = dt.bfloat16
    AF = mybir.ActivationFunctionType
    OP = mybir.AluOpType

    nc = bacc.Bacc(num_devices=NCORE)

    # ---- I/O ----
    x2d_d = nc.dram_tensor("x2d", [E, N], f32, kind="ExternalInput")
    wbias_d = nc.dram_tensor("wbias", [128, 432], f32, kind="ExternalInput")
    ones_d = nc.dram_tensor("onesblk", [128, 9], f32, kind="ExternalInput")
    onesrow_d = nc.dram_tensor("onesrow", [1, 128], f32, kind="ExternalInput")
    lnw_d = nc.dram_tensor("lnw_neg_t", [3, S, E], f32, kind="ExternalInput")
    lnb_d = nc.dram_tensor("lnb_t", [3, S, E], f32, kind="ExternalInput")
    olnw_d = nc.dram_tensor("olnw_bc", [128, E], f32, kind="ExternalInput")
    olnb_d = nc.dram_tensor("olnb_bc", [128, E], f32, kind="ExternalInput")
    fcwt_d = nc.dram_tensor("fcwt", [E, E], f32, kind="ExternalInput")
    fcb_d = nc.dram_tensor("fcb_bc", [128, E], f32, kind="ExternalInput")
    eyeq_d = nc.dram_tensor("eyeq", [128, 394], f32, kind="ExternalInput")
    cst_d = nc.dram_tensor("cstcol", [128, 2], f32, kind="ExternalInput")
    out_d = nc.dram_tensor("out", [S, E], f32, kind="ExternalOutput")

    # internal DRAM
    ybuf = [nc.dram_tensor(f"ybuf{p}", [RPC * N], f32) for p in range(3)]

    with ExitStack() as ctx:
        tc = ctx.enter_context(tile.TileContext(nc))
        const = ctx.enter_context(tc.tile_pool(name="const", bufs=1))

        wb = const.tile([128, 432], f32)
        nc.sync.dma_start(wb[:], wbias_d[:])
        ones = const.tile([128, 9], f32)
        nc.sync.dma_start(ones[:], ones_d[:])
        onesrow = const.tile([1, 128], f32)
        nc.sync.dma_start(onesrow[:], onesrow_d[:])
        eyeq = const.tile([128, 394], f32)
        nc.sync.dma_start(eyeq[:], eyeq_d[:])
        cst = const.tile([128, 2], f32)
        nc.sync.dma_start(cst[:], cst_d[:])

        ccs = const.tile([1, 8], f32)
        cco = const.tile([1, 8], f32)
        nc.vector.memset(ccs[:], 0.0)

        # bf16 copy of the +/- block-ones stationary matrices (bf16 matmul
        # runs at 1 cycle/row on the PE; exact for 0/+-1 values)
        ones_r = const.tile([128, 8], bf16)
        nc.vector.tensor_copy(ones_r[:], ones[:, 0:8])

        # ================= Phase A: adder projections =================
        with ExitStack() as actx:
            xrep_pool = actx.enter_context(tc.tile_pool(name="xrep", bufs=1))
            xrep = xrep_pool.tile([128, 12 * N], f32)
            for r in range(12):
                for j in range(4):
                    nc.sync.dma_start(
                        xrep[32 * j:32 * j + 32, r * N:(r + 1) * N],
                        x2d_d[32 * r:32 * r + 32, :],
                    )

            tmp_pool = actx.enter_context(tc.tile_pool(name="tmp", bufs=13))
            zpool = actx.enter_context(tc.tile_pool(name="zneg", bufs=3))
            evac_pool = actx.enter_context(tc.tile_pool(name="evac", bufs=4))
            psA = actx.enter_context(
                tc.tile_pool(name="psA", bufs=2, space="PSUM"))

            # chunk -> producer assignment, repeating mod 16:
            # 'A' = ACT fused Abs, 'P' = DVE relu-pair, 'E' = DVE 2-op abs
            PAT = "APAPAEAPAPAEAPAA"

            for g in range(36):
                ps = psA.tile([4, N], f32)
                started = [False] * 4
                for r in range(12):
                    idx = g * 12 + r
                    kind = PAT[idx % 16]
                    xin = xrep[:, r * N:(r + 1) * N]
                    wcol = wb[:, idx:idx + 1]
                    outs = []   # (tile, lhsT-slice)
                    if kind == "A":
                        t = tmp_pool.tile([128, N], bf16, tag="tmp")
                        nc.scalar.activation(t[:], xin, AF.Abs, bias=wcol)
                        outs.append((t, ones_r[:, 0:4]))
                    elif kind == "P":
                        t = tmp_pool.tile([128, N], bf16, tag="tmp")
                        nc.vector.tensor_scalar(
                            t[:], xin, wcol, 0.0, OP.add, OP.max)
                        t2 = tmp_pool.tile([128, N], bf16, tag="tmp")
                        nc.vector.tensor_scalar(
                            t2[:], xin, wcol, 0.0, OP.add, OP.min)
                        outs.append((t, ones_r[:, 0:4]))
                        outs.append((t2, ones_r[:, 4:8]))
                    else:
                        z = zpool.tile([128, N], f32, tag="zneg")
                        nc.vector.tensor_scalar(
                            z[:], xin, wcol, -1.0, OP.add, OP.mult)
                        t = tmp_pool.tile([128, N], bf16, tag="tmp")
                        nc.vector.scalar_tensor_tensor(
                            t[:], xin, wcol, z[:], OP.add, OP.max)
                        outs.append((t, ones_r[:, 0:4]))
                    last = (r == 11)
                    for ti, (t, lh) in enumerate(outs):
                        tlast = last and ti == len(outs) - 1
                        for ci_, (a, b) in enumerate(NCH):
                            nc.tensor.matmul(
                                ps[:, a:b], lh, t[:, a:b],
                                start=not started[ci_], stop=tlast)
                            started[ci_] = True
                p_, g_ = g // 12, g % 12
                ev = evac_pool.tile([4, N], f32, tag="evac")
                if g % 2 == 0:
                    nc.scalar.copy(ev[:], ps[:])
                else:
                    nc.vector.tensor_copy(ev[:], ps[:])
                nc.sync.dma_start(
                    ybuf[p_][g_ * 4 * N:(g_ + 1) * 4 * N].rearrange(
                        "(a b) -> a b", b=N),
                    ev[:])

        # ================= Stats + AllReduce =================
        FST = RPC * N // 128  # 591
        with ExitStack() as sctx:
            stp = sctx.enter_context(tc.tile_pool(name="stats", bufs=3))
            psS = sctx.enter_context(
                tc.tile_pool(name="psS", bufs=2, space="PSUM"))
            for p in range(3):
                ys = stp.tile([128, FST], f32, tag="ys")
                nc.sync.dma_start(
                    ys[:], ybuf[p][:].rearrange("(a b) -> a b", b=FST))
                junk = stp.tile([128, FST], f32, tag="junk")
                junk2 = stp.tile([128, FST], f32, tag="junk2")
                spp = stp.tile([128, 2], f32, tag="spp")
                nc.vector.tensor_scalar(
                    junk[:], ys[:], 1.0, None, OP.mult, OP.add,
                    accum_out=spp[:, 0:1])
                nc.scalar.activation(
                    junk2[:], ys[:], AF.Square, bias=cst[:, 0:1],
                    accum_out=spp[:, 1:2])
                pr = psS.tile([1, 2], f32, tag="pr")
                nc.tensor.matmul(pr[:], ones[:, 8:9], spp[:],
                                 start=True, stop=True)
                nc.scalar.copy(ccs[0:1, 2 * p:2 * p + 2], pr[:])

            ccdram = sctx.enter_context(
                tc.tile_pool(name="ccdram", bufs=1, space="DRAM"))
            cc_in_t = ccdram.tile([1, 8], f32, name="cc_in_t")
            cc_out_t = ccdram.tile([1, 8], f32, name="cc_out_t")
            nc.gpsimd.dma_start(cc_in_t[:], ccs[:])
            nc.gpsimd.collective_compute(
                "AllReduce", mybir.AluOpType.add,
                replica_groups=[list(range(NCORE))],
                ins=[cc_in_t.opt()], outs=[cc_out_t.opt()])
            nc.gpsimd.dma_start(cco[:], cc_out_t[:])

            # broadcast gathered stats to all partitions: [1,8] -> [128,8]
            psb = psS.tile([128, 8], f32, tag="psb")
            nc.tensor.matmul(psb[:], onesrow[:], cco[:], start=True, stop=True)
            statb = const.tile([128, 8], f32)
            nc.scalar.copy(statb[:], psb[:])

            # per-proj scalars (uniform across partitions):
            # mu = S1/NTOT ; m' = mu - C ; var = S2/NTOT - m'^2
            # rs = Rsqrt(var + eps) ; negmu = -mu
            mu3 = const.tile([128, 3], f32)
            nc.vector.tensor_scalar(
                mu3[:], statb[:, 0:5:2], 1.0 / NTOT, None, OP.mult)
            m2r = const.tile([128, 3], f32)
            nc.vector.tensor_scalar(
                m2r[:], statb[:, 1:6:2], 1.0 / NTOT, None, OP.mult)
            mp = const.tile([128, 3], f32)
            nc.vector.tensor_scalar(
                mp[:], mu3[:], -C_SHIFT, None, OP.add)
            mp2 = const.tile([128, 3], f32)
            nc.scalar.activation(mp2[:], mp[:], AF.Square)
            var3 = const.tile([128, 3], f32)
            nc.vector.tensor_tensor(var3[:], m2r[:], mp2[:], OP.subtract)
            sdv = const.tile([128, 3], f32)
            nc.scalar.activation(sdv[:], var3[:], AF.Sqrt, bias=cst[:, 1:2])
            rsv = const.tile([128, 3], f32)
            nc.vector.reciprocal(rsv[:], sdv[:])
            negmu = const.tile([128, 3], f32)
            nc.vector.tensor_scalar(
                negmu[:], mu3[:], -1.0, None, OP.mult)

        # ================= Phase B: LN + attention + out =================
        with ExitStack() as bctx:
            tpool = bctx.enter_context(tc.tile_pool(name="T", bufs=1))
            wpool = bctx.enter_context(tc.tile_pool(name="lnp", bufs=4))
            psB = bctx.enter_context(
                tc.tile_pool(name="psB", bufs=1, space="PSUM"))
            sb = bctx.enter_context(tc.tile_pool(name="sb", bufs=6))

            # --- LN-apply: T = A*W2 + B2, W2 = rs*(-lnw), B2 = lnb - mu*W2
            T = {}    # (proj, sblk) -> [sP, 384] token-major tiles
            for p in range(3):
                for si, (s0, s1) in enumerate(SBLK):
                    sP = s1 - s0
                    yt = wpool.tile([sP, E], f32, tag="yt")
                    nc.sync.dma_start(
                        yt[:],
                        ybuf[p][s0 * E:s1 * E].rearrange("(a b) -> a b", b=E))
                    lw = wpool.tile([sP, E], f32, tag="lw")
                    nc.sync.dma_start(lw[:], lnw_d[p, s0:s1, :])
                    lb = wpool.tile([sP, E], f32, tag="lb")
                    nc.sync.dma_start(lb[:], lnb_d[p, s0:s1, :])
                    w2 = wpool.tile([sP, E], f32, tag="w2")
                    nc.vector.tensor_scalar(
                        w2[:], lw[:], rsv[0:sP, p:p + 1], None, OP.mult)
                    b2 = wpool.tile([sP, E], f32, tag="b2")
                    nc.vector.scalar_tensor_tensor(
                        b2[:], w2[:], negmu[0:sP, p:p + 1], lb[:],
                        OP.mult, OP.add)
                    tt_ = tpool.tile([sP, E], f32, tag=f"T{p}{si}")
                    nc.vector.tensor_tensor(w2[:], yt[:], w2[:], OP.mult)
                    nc.vector.tensor_tensor(tt_[:], w2[:], b2[:], OP.add)
                    T[(p, si)] = tt_

            # --- feature-major transposes for q, k: TT[p][eb] = [128, 197]
            TT = {}
            for p in range(2):
                for ei, (e0, e1) in enumerate(EBLK):
                    pst = psB.tile([128, S], f32, tag="pst", bufs=2)
                    for si, (s0, s1) in enumerate(SBLK):
                        sP = s1 - s0
                        nc.tensor.transpose(
                            pst[:, s0:s1],
                            T[(p, si)][:, e0:e1],
                            eyeq[0:sP, 0:sP])
                    tt_ = sb.tile([128, S], f32, tag=f"TT{p}{ei}")
                    nc.scalar.copy(tt_[:], pst[:])
                    TT[(p, ei)] = tt_

            # --- attention per head ---
            o_nat = [tpool.tile([s1 - s0, E], f32, tag=f"on{si}",
                                name=f"on{si}")
                     for si, (s0, s1) in enumerate(SBLK)]
            for h in range(6):
                ei, r0 = (h * D) // 128, (h * D) % 128
                qT = TT[(0, ei)][r0:r0 + D, :]
                kT = TT[(1, ei)][r0:r0 + D, :]
                att = []
                for si, (s0, s1) in enumerate(SBLK):
                    sP = s1 - s0
                    sc = psB.tile([sP, S], f32, tag="sc", bufs=2)
                    nc.tensor.matmul(sc[:], qT[:, s0:s1], kT[:],
                                     start=True, stop=True)
                    pexp = sb.tile([sP, S], f32, tag="pexp")
                    rsum = sb.tile([sP, 1], f32, tag="rsum")
                    nc.scalar.activation(
                        pexp[:], sc[:], AF.Exp, scale=SCALE,
                        accum_out=rsum[:])
                    rinv = sb.tile([sP, 1], f32, tag="rinv")
                    nc.vector.reciprocal(rinv[:], rsum[:])
                    at = sb.tile([sP, S], f32, tag="at")
                    nc.vector.scalar_tensor_tensor(
                        at[:], pexp[:], rinv[:],
                        eyeq[0:sP, 197 * si:197 * si + S],
                        OP.mult, OP.add)
                    att.append(at)
                # transpose att -> attT tiles [tP, 197]
                attT = []
                for ti, (t0, t1) in enumerate(SBLK):
                    tP = t1 - t0
                    pat = psB.tile([tP, S], f32, tag="pat", bufs=2)
                    for si, (s0, s1) in enumerate(SBLK):
                        sP = s1 - s0
                        nc.tensor.transpose(
                            pat[:, s0:s1], att[si][:, t0:t1],
                            eyeq[0:sP, 0:sP])
                    at_ = sb.tile([tP, S], f32, tag="atT")
                    nc.scalar.copy(at_[:], pat[:])
                    attT.append(at_)
                # o = att @ v : out[s,d] = sum_t attT[t,s] v[t,d]
                for si, (s0, s1) in enumerate(SBLK):
                    sP = s1 - s0
                    ops_ = psB.tile([sP, D], f32, tag="ops")
                    for ti, (t0, t1) in enumerate(SBLK):
                        nc.tensor.matmul(
                            ops_[:],
                            attT[ti][:, s0:s1],
                            T[(2, ti)][:, h * D:(h + 1) * D],
                            start=(ti == 0), stop=(ti == 1))
                    nc.scalar.copy(o_nat[si][:, h * D:(h + 1) * D], ops_[:])

            # --- token-local LayerNorm on o + fc ---
            olnw = const.tile([128, E], f32)
            nc.sync.dma_start(olnw[:], olnw_d[:])
            olnb = const.tile([128, E], f32)
            nc.sync.dma_start(olnb[:], olnb_d[:])
            fcb = const.tile([128, E], f32)
            nc.sync.dma_start(fcb[:], fcb_d[:])
            fcw = []
            for ei, (e0, e1) in enumerate(EBLK):
                fw = const.tile([128, E], f32, tag=f"fw{ei}",
                                name=f"fw{ei}")
                nc.sync.dma_start(fw[:], fcwt_d[e0:e1, :])
                fcw.append(fw)

            oln = []
            for si, (s0, s1) in enumerate(SBLK):
                sP = s1 - s0
                on = o_nat[si]
                junk = sb.tile([sP, E], f32, tag="junkB")
                junk2 = sb.tile([sP, E], f32, tag="junkB2")
                os1 = sb.tile([sP, 1], f32, tag="os1")
                os2 = sb.tile([sP, 1], f32, tag="os2")
                nc.vector.tensor_scalar(
                    junk[:], on[:], 1.0, None, OP.mult, OP.add,
                    accum_out=os1[:])
                nc.scalar.activation(
                    junk2[:], on[:], AF.Square, accum_out=os2[:])
                muo = sb.tile([sP, 1], f32, tag="muo")
                nc.vector.tensor_scalar(
                    muo[:], os1[:], 1.0 / E, None, OP.mult)
                m2o = sb.tile([sP, 1], f32, tag="m2o")
                nc.vector.tensor_scalar(
                    m2o[:], os2[:], 1.0 / E, None, OP.mult)
                mu2o = sb.tile([sP, 1], f32, tag="mu2o")
                nc.scalar.activation(mu2o[:], muo[:], AF.Square)
                varo = sb.tile([sP, 1], f32, tag="varo")
                nc.vector.tensor_tensor(varo[:], m2o[:], mu2o[:], OP.subtract)
                sdo = sb.tile([sP, 1], f32, tag="sdo")
                nc.scalar.activation(
                    sdo[:], varo[:], AF.Sqrt, bias=cst[0:sP, 1:2])
                rso = sb.tile([sP, 1], f32, tag="rso")
                nc.vector.reciprocal(rso[:], sdo[:])
                nmuo = sb.tile([sP, 1], f32, tag="nmuo")
                nc.vector.tensor_scalar(nmuo[:], muo[:], -1.0, None, OP.mult)
                z = sb.tile([sP, E], f32, tag="z")
                nc.vector.tensor_scalar(
                    z[:], on[:], nmuo[:], rso[:], OP.add, OP.mult)
                ol = sb.tile([sP, E], f32, tag="ol")
                nc.vector.tensor_tensor(z[:], z[:], olnw[0:sP, :], OP.mult)
                nc.vector.tensor_tensor(ol[:], z[:], olnb[0:sP, :], OP.add)
                oln.append(ol)

            # transpose oln -> [384, 197] feature-major for fc lhsT
            olnT = []
            for ei, (e0, e1) in enumerate(EBLK):
                pst = psB.tile([128, S], f32, tag="pst", bufs=2)
                for si, (s0, s1) in enumerate(SBLK):
                    sP = s1 - s0
                    nc.tensor.transpose(
                        pst[:, s0:s1], oln[si][:, e0:e1], eyeq[0:sP, 0:sP])
                ot = sb.tile([128, S], f32, tag=f"olnT{ei}")
                nc.scalar.copy(ot[:], pst[:])
                olnT.append(ot)

            for si, (s0, s1) in enumerate(SBLK):
                sP = s1 - s0
                fps = psB.tile([sP, E], f32, tag="fps")
                for ei in range(3):
                    nc.tensor.matmul(
                        fps[:], olnT[ei][:, s0:s1], fcw[ei][:],
                        start=(ei == 0), stop=(ei == 2))
                fin = sb.tile([sP, E], f32, tag="fin")
                nc.vector.scalar_tensor_tensor(
                    fin[:], fps[:], 1.0, fcb[0:sP, :], OP.mult, OP.add)
                nc.sync.dma_start(out_d[s0:s1, :], fin[:])

    nc.compile()
    return nc


def _prep_inputs(inputs):
    """Build the 8 per-core input maps from full inputs."""
    x = np.ascontiguousarray(np.asarray(inputs["x"], dtype=np.float32))
    x2d = x.reshape(E, N)
    wq = np.asarray(inputs["wq"], dtype=np.float32)
    wk = np.asarray(inputs["wk"], dtype=np.float32)
    wv = np.asarray(inputs["wv"], dtype=np.float32)
    lnw = [np.asarray(inputs[k], dtype=np.float32).reshape(E, N)
           for k in ("qln_w", "kln_w", "vln_w")]
    lnb = [np.asarray(inputs[k], dtype=np.float32).reshape(E, N)
           for k in ("qln_b", "kln_b", "vln_b")]
    oln_w = np.asarray(inputs["oln_w"], dtype=np.float32)
    oln_b = np.asarray(inputs["oln_b"], dtype=np.float32)
    fc_w = np.asarray(inputs["fc_w"], dtype=np.float32)
    fc_b = np.asarray(inputs["fc_b"], dtype=np.float32)

    onesblk = np.zeros((128, 9), np.float32)
    for j in range(4):
        onesblk[32 * j:32 * j + 32, j] = 1.0
        onesblk[32 * j:32 * j + 32, 4 + j] = -1.0
    onesblk[:, 8] = 1.0
    onesrow = np.ones((1, 128), np.float32)
    eyeq = np.zeros((128, 394), np.float32)
    ey = np.eye(S, dtype=np.float32)
    eyeq[:, 0:S] = ey[0:128]
    eyeq[0:69, 197:394] = ey[128:]
    olnw_bc = np.ascontiguousarray(
        np.broadcast_to(oln_w, (128, E)).astype(np.float32))
    olnb_bc = np.ascontiguousarray(
        np.broadcast_to(oln_b, (128, E)).astype(np.float32))
    fcb_bc = np.ascontiguousarray(
        np.broadcast_to(fc_b, (128, E)).astype(np.float32))
    fcwt = np.ascontiguousarray(fc_w.T)
    cstcol = np.zeros((128, 2), np.float32)
    cstcol[:, 0] = -C_SHIFT
    cstcol[:, 1] = EPS

    in_maps = []
    for c in range(NCORE):
        sl = slice(c * RPC, (c + 1) * RPC)
        w_core = np.concatenate([wq[sl], wk[sl], wv[sl]], axis=0)  # [144,384]
        A = (-w_core).reshape(36, 4, 12, 32)
        wbias = np.ascontiguousarray(
            A.transpose(1, 3, 0, 2).reshape(128, 432))
        lnw_neg_t = np.stack(
            [(-m[sl]).reshape(S, E) for m in lnw]).astype(np.float32)
        lnb_t = np.stack(
            [m[sl].reshape(S, E) for m in lnb]).astype(np.float32)
        in_maps.append({
            "x2d": x2d,
            "wbias": wbias,
            "onesblk": onesblk,
            "onesrow": onesrow,
            "lnw_neg_t": np.ascontiguousarray(lnw_neg_t),
            "lnb_t": np.ascontiguousarray(lnb_t),
            "olnw_bc": olnw_bc,
            "olnb_bc": olnb_bc,
            "fcwt": fcwt,
            "fcb_bc": fcb_bc,
            "eyeq": eyeq,
            "cstcol": cstcol,
        })
    return in_maps


def get_program():
    global _PROGRAM
    if _PROGRAM is None:
        _PROGRAM = _build_program()
    return _PROGRAM


def kernel(**inputs):
    from concourse.bass_utils import run_bass_kernel_spmd
    nc = get_program()
    in_maps = _prep_inputs(inputs)
    res = run_bass_kernel_spmd(nc, in_maps, list(range(NCORE)))
    out = np.stack([res.results[c]["out"] for c in range(NCORE)])
    return out.astype(np.float32)
